# revision 7
# baseline (speedup 1.0000x reference)
"""Sparse-attention (graph-modulated MHA) Bass kernel for Trainium2.

Strategy: data-parallel over batch (8 batches -> 8 NeuronCores). Per core:
  - key/value positions compressed host-side using the key mask: positions
    0..99 stay in place (graph block alignment), unmasked positions >=100
    gathered behind them, rest padded with -1e9 exp-bias => KV=320 slots
    instead of 512 (cuts K-proj, scores and att@V PE work)
  - bf16 matmuls (fp32 psum); V projection runs k-outer (one psum
    accumulator per (kv-chunk, half)) so it starts on the first DMA chunk;
    Q/K projections per-pair inside the attention loop with per-pair
    weight blocks (host pre-arranged) so DMA priority matches first use
  - scores computed transposed sT[k_pos, q]; two heads share one
    [128, 2*512] psum tile; graph block multiplied on raw fp32 psum scores;
    key mask folded into the exp bias (per-partition bias in this layout)
  - softmax without max-subtraction; denominator L from an extra ones-col
    in the attention*V matmul (both heads in one [65, 2*512] psum tile);
    L row copied to partition 0 (DVE), one fast reciprocal, broadcast via
    gpsimd partition_broadcast, applied fused with the psum->sbuf eviction
  - merge projection emitted transposed, bf16 out; host transposes back
"""
import sys

sys.path.insert(0, "/opt/trn_rl_repo")

import ml_dtypes
import numpy as np

import concourse.bass as bass
import concourse.mybir as mybir
import concourse.tile as tile
from concourse import bacc, bass_utils
from concourse.bass import ds, ts

B, S, D, H, DK = 8, 512, 1024, 16, 64
GN = 100
P = 128
NDT = D // P      # 8 hidden chunks of 128
NPAIR = H // 2    # 8 head pairs (2 heads share a 128-partition tile)
EH = DK + 1       # head slot width in vha (64 v-cols + 1 ones col)
F32 = mybir.dt.float32
BF16 = mybir.dt.bfloat16
FT = mybir.ActivationFunctionType
ALU = mybir.AluOpType

_CACHE: dict = {}


def _build_module(KV):
    NKC = (KV + P - 1) // P
    CSZ = [min(P, KV - c * P) for c in range(NKC)]

    nc = bacc.Bacc("TRN2", target_bir_lowering=False, debug=False)
    dram = {}
    dram["qinT"] = nc.dram_tensor("qinT", [D, S], BF16, kind="ExternalInput").ap()
    for nm in ("kinT", "vinT"):
        dram[nm] = nc.dram_tensor(nm, [D, KV], BF16, kind="ExternalInput").ap()
    dram["wvT"] = nc.dram_tensor("wvT", [D, D], BF16, kind="ExternalInput").ap()
    # per-pair blocks: row m*128+p, col k*128+j  ->  W^T[k*128+p, m*128+j]
    for nm in ("wqP", "wkP", "wmP"):
        dram[nm] = nc.dram_tensor(nm, [D, D], BF16, kind="ExternalInput").ap()
    for nm in ("bq", "bk", "bm"):
        dram[nm] = nc.dram_tensor(nm, [P, NDT], F32, kind="ExternalInput").ap()
    dram["bv"] = nc.dram_tensor("bv", [1, D], F32, kind="ExternalInput").ap()
    dram["maskb"] = nc.dram_tensor("maskb", [P, NKC], F32, kind="ExternalInput").ap()
    dram["gT"] = nc.dram_tensor("gT", [GN, GN], F32, kind="ExternalInput").ap()
    outT = nc.dram_tensor("outT", [D, S], BF16, kind="ExternalOutput").ap()

    with tile.TileContext(nc) as tc:
        with (
            tc.tile_pool(name="wvpool", bufs=8) as wvpool,
            tc.tile_pool(name="vxpool", bufs=8) as vxpool,
            tc.tile_pool(name="xbig", bufs=1) as xbig,
            tc.tile_pool(name="wppool", bufs=16) as wppool,
            tc.tile_pool(name="qkpool", bufs=8) as qkpool,
            tc.tile_pool(name="vpool", bufs=3) as vpool,
            tc.tile_pool(name="ptpool", bufs=9) as ptpool,
            tc.tile_pool(name="opool", bufs=8) as opool,
            tc.tile_pool(name="outpool", bufs=3) as outpool,
            tc.tile_pool(name="cpool", bufs=1) as cpool,
            tc.tile_pool(name="rlpool", bufs=2) as rlpool,
            tc.tile_pool(name="lbcpool", bufs=2) as lbcpool,
            tc.tile_pool(name="spsum", bufs=2, space="PSUM") as spsum,
            tc.tile_pool(name="apsum", bufs=2, space="PSUM") as apsum,
        ):
            # ---- constants (scalar DMA queue; small, early) ----
            bqt = cpool.tile([P, NDT], F32, tag="bqt")
            nc.sync.dma_start(bqt[:], dram["bq"])
            bkt = cpool.tile([P, NDT], F32, tag="bkt")
            nc.gpsimd.dma_start(bkt[:], dram["bk"])
            bmt = cpool.tile([P, NDT], F32, tag="bmt")
            nc.sync.dma_start(bmt[:], dram["bm"])
            maskbt = cpool.tile([P, NKC], F32, tag="maskbt")
            nc.gpsimd.dma_start(maskbt[:], dram["maskb"])
            gt = cpool.tile([P, GN], F32, tag="gt")
            nc.sync.dma_start(gt[0:GN, :], dram["gT"])
            bvb = cpool.tile([P, D], F32, tag="bvb")
            nc.gpsimd.dma_start(bvb[:], dram["bv"].to_broadcast((P, D)))

            # PE warmup: fine-grained matmuls on memset tiles while the first
            # DMAs land, so the HAM un-throttles before real matmuls
            warm_w = cpool.tile([P, DK], BF16, tag="warmw")
            nc.gpsimd.memset(warm_w[:], 0.0)
            warm_x = cpool.tile([P, 256], BF16, tag="warmx")
            nc.gpsimd.memset(warm_x[:], 0.0)
            wps = apsum.tile([EH, 2 * S], F32, tag="ap", name="warmps")
            for _ in range(24):
                nc.tensor.matmul(wps[0:DK, 0:256], warm_w[:], warm_x[:], start=True, stop=True)

            # ---- input streaming, in first-use priority order ----
            def big_load(name, width):
                t_ = xbig.tile([P, NDT * width], BF16, tag=name, name=name)
                src = dram[name].rearrange("(t p) f -> p t f", p=P)
                dst = t_.rearrange("p (t f) -> p t f", f=width)
                nc.sync.dma_start(dst[:, 0 : NDT // 2], src[:, 0 : NDT // 2])
                nc.gpsimd.dma_start(dst[:, NDT // 2 : NDT], src[:, NDT // 2 : NDT])
                return t_

            wq, wk, wm = [None] * NPAIR, [None] * NPAIR, [None] * NDT
            wq_src = dram["wqP"].rearrange("(m p) f -> m p f", p=P)
            wk_src = dram["wkP"].rearrange("(m p) f -> m p f", p=P)

            def load_pair_w(t):
                w1 = wppool.tile([P, D], BF16, tag="wp", name=f"wq{t}")
                nc.sync.dma_start(w1[:], wq_src[t])
                wq[t] = w1
                w2 = wppool.tile([P, D], BF16, tag="wp", name=f"wk{t}")
                nc.gpsimd.dma_start(w2[:], wk_src[t])
                wk[t] = w2

            qtb = big_load("qinT", S)
            load_pair_w(0)
            ktb = big_load("kinT", KV)
            load_pair_w(1)
            load_pair_w(2)
            wvt, vt = [], []
            wv_src = dram["wvT"].rearrange("(t p) f -> t p f", p=P)
            v_src = dram["vinT"].rearrange("(t p) f -> t p f", p=P)
            for k in range(NDT):
                t_ = wvpool.tile([P, D], BF16, tag="wv", name=f"wvt{k}")
                nc.sync.dma_start(t_[:], wv_src[k])
                wvt.append(t_)
                t_ = vxpool.tile([P, KV], BF16, tag="vx", name=f"vt{k}")
                nc.gpsimd.dma_start(t_[:], v_src[k])
                vt.append(t_)
            for t in range(3, NPAIR):
                load_pair_w(t)
            wm_src = dram["wmP"].rearrange("(m p) f -> m p f", p=P)
            for m in range(NDT):
                w3 = wppool.tile([P, D], BF16, tag="wm", name=f"wm{m}")
                (nc.sync if m % 2 == 0 else nc.gpsimd).dma_start(w3[:], wm_src[m])
                wm[m] = w3

            # ---- V projection: k-outer over 6 psum accumulators ----
            vha = [vpool.tile([P, H * EH], BF16, tag="vha", name=f"vha{c}") for c in range(NKC)]

            def emit_vproj():
              sp1 = spsum.tile([P, 2 * S], F32, tag="sp", name="vsp1")
              sp2 = spsum.tile([P, 2 * S], F32, tag="sp", name="vsp2")
              apv = apsum.tile([P, 2 * S], F32, tag="ap", name="vap")
              vacc = {
                  (0, 0): sp1[:, 0:S],
                  (0, 1): sp1[:, S : 2 * S],
                  (0, 2): sp2[:, 0:S],
                  (1, 0): sp2[:, S : 2 * S],
                  (1, 1): apv[:, 0:S],
                  (1, 2): apv[:, S : 2 * S],
              }
              for k in range(NDT):
                  for c in range(NKC):
                      csz = CSZ[c]
                      for half in range(2):
                          nc.tensor.matmul(
                              vacc[(half, c)][0:csz, :],
                              vt[k][:, ds(c * P, csz)],
                              wvt[k][:, ts(half, 512)],
                              start=(k == 0), stop=(k == NDT - 1),
                          )
              for c in range(NKC):
                  csz = CSZ[c]
                  v3 = vha[c].rearrange("p (h e) -> p h e", e=EH)
                  for half in range(2):
                      src3 = vacc[(half, c)][0:csz, :].rearrange("p (h d) -> p h d", d=DK)
                      bv3 = bvb[0:csz, ts(half, 512)].rearrange("p (h d) -> p h d", d=DK)
                      nc.vector.tensor_tensor(
                          v3[0:csz, half * 8 : half * 8 + 8, 0:DK], src3, bv3, ALU.add
                      )
                  nc.vector.memset(v3[0:csz, :, DK:EH], 1.0)
              for _ in range(6):
                  nc.tensor.matmul(wps[0:DK, 0:256], warm_w[:], warm_x[:], start=True, stop=True)

            # ---- per-pair Q/K projections + scores ----
            qT, kT = [None] * NPAIR, [None] * NPAIR

            def emit_qkproj(t):
                spqk = spsum.tile([P, 2 * S], F32, tag="sp", name=f"qk{t}")
                for k in range(NDT):
                    nc.tensor.matmul(
                        spqk[:, 0:S], wq[t][:, ds(k * P, P)], qtb[:, ds(k * S, S)],
                        start=(k == 0), stop=(k == NDT - 1),
                    )
                t1 = qkpool.tile([P, S], BF16, tag="q", name=f"qT{t}")
                nc.scalar.activation(t1[:], spqk[:, 0:S], FT.Identity, bias=bqt[:, t : t + 1])
                qT[t] = t1
                for k in range(NDT):
                    nc.tensor.matmul(
                        spqk[:, S : S + KV], wk[t][:, ds(k * P, P)], ktb[:, ds(k * KV, KV)],
                        start=(k == 0), stop=(k == NDT - 1),
                    )
                t2 = qkpool.tile([P, KV], BF16, tag="k", name=f"kT{t}")
                nc.scalar.activation(t2[:], spqk[:, S : S + KV], FT.Identity, bias=bkt[:, t : t + 1])
                kT[t] = t2

            def emit_scores(t):
                tiles = [None] * NKC
                for c in range(NKC):
                    csz = CSZ[c]
                    sps = spsum.tile([P, 2 * S], F32, tag="sp", name=f"sc{t}_{c}")
                    for x in range(2):
                        nc.tensor.matmul(
                            sps[0:csz, ts(x, S)],
                            kT[t][x * DK : (x + 1) * DK, ds(c * P, csz)],
                            qT[t][x * DK : (x + 1) * DK, :],
                            start=True, stop=True,
                        )
                        if c == 0:
                            nc.vector.tensor_tensor(
                                sps[0:GN, x * S : x * S + GN],
                                sps[0:GN, x * S : x * S + GN],
                                gt[0:GN, :], ALU.mult,
                            )
                    pt = ptpool.tile([P, 2 * S], BF16, tag="pt", name=f"pt{t}_{c}")
                    nc.scalar.activation(
                        pt[0:csz, :], sps[0:csz, :], FT.Exp,
                        bias=maskbt[0:csz, c : c + 1], scale=0.125,
                    )
                    tiles[c] = pt
                return tiles

            def emit_av_mms(t, ptiles):
                ops = apsum.tile([EH, 2 * S], F32, tag="ap", name=f"av{t}")
                for x in range(2):
                    h = 2 * t + x
                    for c in range(NKC):
                        csz = CSZ[c]
                        nc.tensor.matmul(
                            ops[:, ts(x, S)], vha[c][0:csz, ds(h * EH, EH)],
                            ptiles[c][0:csz, ts(x, S)],
                            start=(c == 0), stop=(c == NKC - 1),
                        )
                lrec = rlpool.tile([1, 2 * S], F32, tag="lrec", name=f"lrec{t}")
                nc.vector.tensor_copy(lrec[0:1, :], ops[DK : DK + 1, :])
                lrr = rlpool.tile([1, 2 * S], F32, tag="lrr", name=f"lrr{t}")
                nc.vector.reciprocal_approx_fast(lrr[0:1, :], lrec[0:1, :])
                lbc = lbcpool.tile([DK, 2 * S], F32, tag="lbc", name=f"lbc{t}")
                nc.gpsimd.partition_broadcast(lbc[:], lrr[0:1, :])
                return ops, lbc

            def emit_av_norm(t, ops, lbc):
                o = opool.tile([P, S], BF16, tag="o", name=f"oT{t}")
                nc.vector.tensor_tensor(
                    o[0:DK, :], ops[0:DK, 0:S], lbc[:, 0:S], ALU.mult
                )
                nc.vector.tensor_tensor(
                    o[DK:P, :], ops[0:DK, S : 2 * S], lbc[:, S : 2 * S], ALU.mult
                )
                oT[t] = o

            oT = [None] * NPAIR

            def emit_av(t, ptiles):
                ops, lbc = emit_av_mms(t, ptiles)
                emit_av_norm(t, ops, lbc)

            # ---- merge helpers ----
            out_view = outT.rearrange("(t p) f -> t p f", p=P)
            mps = {}

            def merge_start2(m):
                """Start merges m and m+1 sharing one [P, 2S] psum tile."""
                sp = spsum.tile([P, 2 * S], F32, tag="sp", name=f"mps{m}")
                mps[m] = sp[:, 0:S]
                mps[m + 1] = sp[:, S : 2 * S]
                for mm in (m, m + 1):
                    for k in range(NDT - 2):
                        nc.tensor.matmul(
                            mps[mm], wm[mm][:, ds(k * P, P)], oT[k][:],
                            start=(k == 0), stop=False,
                        )

            def merge_fin(m):
                ps = mps.pop(m)
                for k in (NDT - 2, NDT - 1):
                    nc.tensor.matmul(
                        ps, wm[m][:, ds(k * P, P)], oT[k][:],
                        start=False, stop=(k == NDT - 1),
                    )
                ot = outpool.tile([P, S], BF16, tag="out", name=f"ot{m}")
                if m % 2 == 0:
                    nc.scalar.activation(ot[:], ps, FT.Identity, bias=bmt[:, m : m + 1])
                else:
                    nc.vector.tensor_scalar_add(ot[:], ps, bmt[:, m : m + 1])
                out_eng = (nc.gpsimd, nc.sync, nc.scalar)[m % 3]
                out_eng.dma_start(out_view[m], ot[:])

            # ---- main interleaved flow: pairs 0-1, V-proj, lag-2 loop ----
            pts = {}
            emit_qkproj(0)
            pts[0] = emit_scores(0)
            emit_qkproj(1)
            pts[1] = emit_scores(1)
            emit_vproj()
            chains = {}
            for t in range(2, NPAIR):
                emit_qkproj(t)
                pts[t] = emit_scores(t)
                chains[t - 2] = emit_av_mms(t - 2, pts.pop(t - 2))
                if t >= 3:
                    emit_av_norm(t - 3, *chains.pop(t - 3))
            # tail: AV(6)/AV(7) chains overlapped with merge matmuls
            chains[6] = emit_av_mms(6, pts.pop(6))
            emit_av_norm(5, *chains.pop(5))
            chains[7] = emit_av_mms(7, pts.pop(7))
            emit_av_norm(6, *chains.pop(6))
            merge_start2(0)
            emit_av_norm(7, *chains.pop(7))
            merge_start2(2)
            for m in range(NDT):
                merge_fin(m)
                if m % 2 == 1 and m + 3 < NDT:
                    merge_start2(m + 3)

    nc.compile()
    return nc


def _get_module(KV):
    key = ("nc", KV)
    if key not in _CACHE:
        _CACHE[key] = _build_module(KV)
    return _CACHE[key]


def _bf16(x: np.ndarray) -> np.ndarray:
    return np.ascontiguousarray(x, dtype=np.float32).astype(ml_dtypes.bfloat16)


def _pair_blocks(WT: np.ndarray) -> np.ndarray:
    # [k*128+p, m*128+j] -> [m*128+p, k*128+j]
    return np.ascontiguousarray(
        WT.reshape(NDT, P, NDT, P).transpose(2, 1, 0, 3).reshape(D, D)
    )


def kernel(q, k, v, mask, graph, Wv, bv, Wk, bk, Wq, bq, Wm, bm, _trace=False):
    q = np.asarray(q, np.float32)
    k = np.asarray(k, np.float32)
    v = np.asarray(v, np.float32)
    mask = np.asarray(mask)
    graph = np.asarray(graph, np.float32)

    keep = [np.concatenate([np.arange(GN), GN + np.flatnonzero(~mask[b, 0, 0, GN:])])
            for b in range(B)]
    needed = max(len(ix) for ix in keep)
    KV = next(kv for kv in (320, 384, 448, 512) if kv >= needed)
    NKC = (KV + P - 1) // P
    nc = _get_module(KV)

    shared = {
        "wqP": _bf16(_pair_blocks(np.asarray(Wq, np.float32).T)),
        "wkP": _bf16(_pair_blocks(np.asarray(Wk, np.float32).T)),
        "wmP": _bf16(_pair_blocks(np.asarray(Wm, np.float32).T)),
        "wvT": _bf16(np.asarray(Wv, np.float32).T),
        "bq": np.ascontiguousarray(np.asarray(bq, np.float32).reshape(NDT, P).T),
        "bk": np.ascontiguousarray(np.asarray(bk, np.float32).reshape(NDT, P).T),
        "bm": np.ascontiguousarray(np.asarray(bm, np.float32).reshape(NDT, P).T),
        "bv": np.asarray(bv, np.float32).reshape(1, D),
    }
    eye = np.eye(GN, dtype=np.float32)
    in_maps = []
    for b in range(B):
        ix = keep[b]
        n = len(ix)
        kg = np.zeros((KV, D), np.float32)
        vg = np.zeros((KV, D), np.float32)
        kg[:n] = k[b][ix]
        vg[:n] = v[b][ix]
        mb = np.full(NKC * P, np.float32(-1e9), np.float32)
        mb[:GN] = np.where(mask[b, 0, 0, :GN], np.float32(-1e9), np.float32(0.0))
        mb[GN:n] = 0.0
        in_maps.append(
            dict(
                shared,
                qinT=_bf16(q[b].T),
                kinT=_bf16(kg.T),
                vinT=_bf16(vg.T),
                maskb=np.ascontiguousarray(mb.reshape(NKC, P).T),
                gT=np.ascontiguousarray((graph[b] + eye).T),
            )
        )

    res = bass_utils.run_bass_kernel_spmd(
        nc, in_maps, core_ids=list(range(B)), trace=_trace
    )
    out = np.stack([np.asarray(r["outT"], ml_dtypes.bfloat16).T.astype(np.float32)
                    for r in res.results])
    if _trace:
        kernel._last_results = res
    return out


# revision 8
# speedup vs baseline: 1.1598x; 1.1598x over previous
"""Sparse-attention (graph-modulated MHA) Bass kernel for Trainium2.

Strategy: data-parallel over batch (8 batches -> 8 NeuronCores). Per core:
  - bf16 matmuls (fp32 psum); V projection first, then Q/K projections
    interleaved per head-pair with the score matmuls so the ACT-engine exp
    work overlaps projection matmuls on the PE
  - scores computed transposed sT[k_pos, q]; the two heads of a pair share
    one [128, 1024] psum tile so one exp covers both; graph block multiplied
    on raw fp32 psum scores; key mask folded into the exp bias
  - softmax without max-subtraction; denominator L from an extra ones-column
    in the attention*V matmul; reciprocals batched (pairs 0-5 mid-loop, rest
    at the end); 1/L broadcast across partitions via DRAM round-trip DMA
  - merge projection emitted transposed (fp32 out); host transposes back
  - bulk loads on the sync DMA queue; small/late DMAs on the gpsimd queue
"""
import sys

sys.path.insert(0, "/opt/trn_rl_repo")

import ml_dtypes
import numpy as np

import concourse.bass as bass
import concourse.mybir as mybir
import concourse.tile as tile
from concourse import bacc, bass_utils
from concourse.bass import ds, ts

B, S, D, H, DK = 8, 512, 1024, 16, 64
GN = 100
P = 128
NKT = S // P      # 4 key-position chunks of 128
NDT = D // P      # 8 hidden chunks of 128
NPAIR = H // 2    # 8 head pairs (2 heads share a 128-partition tile)
EH = DK + 1       # head slot width in vha (64 v-cols + 1 ones col)
NB1 = 6           # pairs normalized in the first (mid-loop) batch
F32 = mybir.dt.float32
BF16 = mybir.dt.bfloat16
FT = mybir.ActivationFunctionType
ALU = mybir.AluOpType

_CACHE: dict = {}


def _build_module():
    nc = bacc.Bacc("TRN2", target_bir_lowering=False, debug=False)
    dram = {}
    for nm in ("qinT", "kinT", "vinT"):
        dram[nm] = nc.dram_tensor(nm, [D, S], BF16, kind="ExternalInput").ap()
    for nm in ("wqT", "wkT", "wvT", "wmT"):
        dram[nm] = nc.dram_tensor(nm, [D, D], BF16, kind="ExternalInput").ap()
    for nm in ("bq", "bk", "bm"):
        dram[nm] = nc.dram_tensor(nm, [P, NDT], F32, kind="ExternalInput").ap()
    dram["bv"] = nc.dram_tensor("bv", [1, D], F32, kind="ExternalInput").ap()
    dram["maskb"] = nc.dram_tensor("maskb", [P, NKT], F32, kind="ExternalInput").ap()
    dram["gT"] = nc.dram_tensor("gT", [GN, GN], F32, kind="ExternalInput").ap()
    outT = nc.dram_tensor("outT", [D, S], F32, kind="ExternalOutput").ap()

    with tile.TileContext(nc) as tc:
        with (
            tc.tile_pool(name="wpool", bufs=24) as wpool,
            tc.tile_pool(name="xpool", bufs=24) as xpool,
            tc.tile_pool(name="qkpool", bufs=16) as qkpool,
            tc.tile_pool(name="vpool", bufs=4) as vpool,
            tc.tile_pool(name="ptpool", bufs=14) as ptpool,
            tc.tile_pool(name="opool", bufs=8) as opool,
            tc.tile_pool(name="outpool", bufs=3) as outpool,
            tc.tile_pool(name="cpool", bufs=1) as cpool,
            tc.tile_pool(name="rlpool", bufs=2) as rlpool,
            tc.tile_pool(name="rlbpool", bufs=4) as rlbpool,
            tc.tile_pool(name="drampool", bufs=2, space="DRAM") as drampool,
            tc.tile_pool(name="ppsum", bufs=2, space="PSUM") as ppsum,
            tc.tile_pool(name="spsum", bufs=2, space="PSUM") as spsum,
            tc.tile_pool(name="apsum", bufs=2, space="PSUM") as apsum,
        ):
            def load_chunks(name, width, eng, eng2=None):
                tiles = []
                src = dram[name].rearrange("(t p) f -> t p f", p=P)
                pool = wpool if width == D else xpool
                for k_i in range(NDT):
                    t_ = pool.tile([P, width], BF16, tag="w" if width == D else "x")
                    e = eng if (eng2 is None or k_i % 2 == 0) else eng2
                    e.dma_start(t_[:], src[k_i])
                    tiles.append(t_)
                return tiles

            # PE warmup: ~4us of full-duty N=512 matmuls on memset tiles while
            # the first DMAs land, so the HAM un-throttles before real matmuls
            warm_w = cpool.tile([P, DK], BF16, tag="warmw")
            nc.vector.memset(warm_w[:], 0.0)
            warm_x = cpool.tile([P, S], BF16, tag="warmx")
            nc.vector.memset(warm_x[:], 0.0)
            wps = apsum.tile([EH, S], F32, tag="ap", name="warmps")
            for _ in range(10):
                nc.tensor.matmul(wps[0:DK, :], warm_w[:], warm_x[:], start=True, stop=True)

            # V inputs stream first (V projection runs first);
            # weights on the sync queue, inputs on the scalar queue
            wvt = load_chunks("wvT", D, nc.sync, nc.gpsimd)
            vt = load_chunks("vinT", S, nc.scalar, nc.sync)

            # ---- constants (gpsimd DMA queue; small) ----
            bqt = cpool.tile([P, NDT], F32, tag="bqt")
            nc.gpsimd.dma_start(bqt[:], dram["bq"])
            bkt = cpool.tile([P, NDT], F32, tag="bkt")
            nc.gpsimd.dma_start(bkt[:], dram["bk"])
            bmt = cpool.tile([P, NDT], F32, tag="bmt")
            nc.gpsimd.dma_start(bmt[:], dram["bm"])
            maskb = cpool.tile([P, NKT], F32, tag="maskb")
            nc.gpsimd.dma_start(maskb[:], dram["maskb"])
            gt = cpool.tile([P, GN], F32, tag="gt")
            nc.gpsimd.dma_start(gt[0:GN, :], dram["gT"])
            bvb = cpool.tile([P, D], F32, tag="bvb")
            nc.gpsimd.dma_start(bvb[:], dram["bv"].to_broadcast((P, D)))
            ones64 = cpool.tile([1, DK], mybir.dt.float32r, tag="ones64")
            nc.vector.memset(ones64[:].bitcast(F32), 1.0)

            # Q/K inputs stream behind V
            wqt = load_chunks("wqT", D, nc.gpsimd)
            qt = load_chunks("qinT", S, nc.scalar)
            wkt = load_chunks("wkT", D, nc.sync)
            ktc = load_chunks("kinT", S, nc.scalar)

            # ---- V projection (natural layout, packed into vha with ones col) ----
            vha = [vpool.tile([P, H * EH], BF16, tag="vha", name=f"vha{i}") for i in range(NKT)]
            for st in range(NKT):
                v3 = vha[st].rearrange("p (h e) -> p h e", e=EH)
                for half in range(2):
                    ps = ppsum.tile([P, S], F32, tag="pp")
                    for k_i in range(NDT):
                        nc.tensor.matmul(
                            ps[:], vt[k_i][:, ts(st, P)], wvt[k_i][:, ts(half, 512)],
                            start=(k_i == 0), stop=(k_i == NDT - 1),
                        )
                    dst3 = v3[:, half * 8 : half * 8 + 8, 0:DK]
                    src3 = ps[:].rearrange("p (h d) -> p h d", d=DK)
                    bv3 = bvb[:, ts(half, 512)].rearrange("p (h d) -> p h d", d=DK)
                    nc.vector.tensor_tensor(dst3, src3, bv3, ALU.add)
                nc.vector.memset(v3[:, :, DK : DK + 1], 1.0)

            # merge weights stream during the attention phase
            wmt = load_chunks("wmT", D, nc.sync)

            # ---- attention state ----
            oT = [opool.tile([P, S], BF16, tag="o", name=f"oT{i}") for i in range(NPAIR)]
            qT, kT = [None] * NDT, [None] * NDT

            def emit_proj(wt, xt, btile, dst, m):
                ps = ppsum.tile([P, S], F32, tag="pp")
                for k_i in range(NDT):
                    nc.tensor.matmul(
                        ps[:], wt[k_i][:, ts(m, P)], xt[k_i][:],
                        start=(k_i == 0), stop=(k_i == NDT - 1),
                    )
                t_ = qkpool.tile([P, S], BF16, tag="qk")
                nc.scalar.activation(
                    t_[:], ps[:], FT.Identity, bias=btile[:, m : m + 1]
                )
                dst[m] = t_

            def emit_scores(t):
                """Both heads of pair t share one [128, 2*S] psum tile per k-chunk."""
                tiles = [None] * NKT
                for kc in range(NKT):
                    sps = spsum.tile([P, 2 * S], F32, tag="sp")
                    for x in range(2):
                        nc.tensor.matmul(
                            sps[:, ts(x, S)],
                            kT[t][x * DK : (x + 1) * DK, ts(kc, P)],
                            qT[t][x * DK : (x + 1) * DK, :],
                            start=True, stop=True,
                        )
                        if kc == 0:
                            nc.vector.tensor_tensor(
                                sps[0:GN, x * S : x * S + GN],
                                sps[0:GN, x * S : x * S + GN],
                                gt[0:GN, :], ALU.mult,
                            )
                    pt = ptpool.tile([P, 2 * S], BF16, tag="pt")
                    nc.scalar.activation(
                        pt[:], sps[:], FT.Exp,
                        bias=maskb[:, kc : kc + 1], scale=0.125,
                    )
                    tiles[kc] = pt
                return tiles

            def emit_av(t, ptiles):
                lrec = rlpool.tile([1, 2 * S], F32, tag="lrec")
                for x in range(2):
                    h = 2 * t + x
                    ops = apsum.tile([EH, S], F32, tag="ap")
                    for kc in range(NKT):
                        nc.tensor.matmul(
                            ops[:], vha[kc][:, ds(h * EH, EH)],
                            ptiles[kc][:, ts(x, S)],
                            start=(kc == 0), stop=(kc == NKT - 1),
                        )
                    lsb_ = rlpool.tile([1, S], F32, tag="lsb")
                    nc.scalar.copy(lsb_[:], ops[DK : DK + 1, :])
                    nc.vector.reciprocal_approx_fast(
                        lrec[0:1, ts(x, S)], lsb_[0:1, :]
                    )
                    nc.vector.tensor_copy(
                        oT[t][x * DK : (x + 1) * DK, :], ops[0:DK, :]
                    )
                rlr = rlbpool.tile([1, 2 * S], mybir.dt.float32r, tag="rlr")
                nc.vector.tensor_copy(rlr[:], lrec[:])
                lba = apsum.tile([EH, S], F32, tag="ap", name=f"lba{t}")
                nc.tensor.matmul(
                    lba[0:DK, :], ones64[:], rlr[0:1, 0:S], start=True, stop=True
                )
                lbb = apsum.tile([EH, S], F32, tag="ap", name=f"lbb{t}")
                nc.tensor.matmul(
                    lbb[0:DK, :], ones64[:], rlr[0:1, ts(1, S)], start=True, stop=True
                )
                oa = oT[t][0:DK, :]
                nc.vector.tensor_tensor(oa, oa, lba[0:DK, :], ALU.mult)
                ob = oT[t][DK:P, :]
                nc.vector.tensor_tensor(ob, ob, lbb[0:DK, :], ALU.mult)

            # ---- merge helpers: kd 0..5 accumulate early, kd 6..7 close late ----
            out_view = outT.rearrange("(t p) f -> t p f", p=P)
            mps = {}

            def merge_start(m):
                if m % 2 == 0:
                    ps = ppsum.tile([P, S], F32, tag="pp", name=f"mps{m}")
                else:
                    ps = spsum.tile([P, 2 * S], F32, tag="sp", name=f"mps{m}")[:, 0:S]
                for k_i in range(NDT - 2):
                    nc.tensor.matmul(
                        ps[:], wmt[k_i][:, ts(m, P)], oT[k_i][:],
                        start=(k_i == 0), stop=False,
                    )
                mps[m] = ps

            def merge_fin(m):
                ps = mps.pop(m)
                for k_i in (NDT - 2, NDT - 1):
                    nc.tensor.matmul(
                        ps[:], wmt[k_i][:, ts(m, P)], oT[k_i][:],
                        start=False, stop=(k_i == NDT - 1),
                    )
                ot = outpool.tile([P, S], F32, tag="out")
                nc.scalar.activation(
                    ot[:], ps[:], FT.Identity, bias=bmt[:, m : m + 1]
                )
                out_eng = (nc.gpsimd, nc.sync, nc.scalar)[m % 3]
                out_eng.dma_start(out_view[m], ot[:])

            # ---- main interleaved loop ----
            prev = None
            for t in range(NPAIR):
                emit_proj(wqt, qt, bqt, qT, t)
                emit_proj(wkt, ktc, bkt, kT, t)
                cur = emit_scores(t)
                if prev is not None:
                    emit_av(t - 1, prev)
                prev = cur
            merge_start(0)
            emit_av(NPAIR - 1, prev)
            merge_start(1)
            merge_start(2)
            merge_start(3)
            for m in range(NDT):
                merge_fin(m)
                if m + 4 < NDT:
                    merge_start(m + 4)

    nc.compile()
    return nc


def _get_module():
    if "nc" not in _CACHE:
        _CACHE["nc"] = _build_module()
    return _CACHE["nc"]


def _bf16(x: np.ndarray) -> np.ndarray:
    return np.ascontiguousarray(x, dtype=np.float32).astype(ml_dtypes.bfloat16)


def kernel(q, k, v, mask, graph, Wv, bv, Wk, bk, Wq, bq, Wm, bm, _trace=False):
    nc = _get_module()
    q = np.asarray(q, np.float32)
    k = np.asarray(k, np.float32)
    v = np.asarray(v, np.float32)
    mask = np.asarray(mask)
    graph = np.asarray(graph, np.float32)

    shared = {
        "wqT": _bf16(np.asarray(Wq, np.float32).T),
        "wkT": _bf16(np.asarray(Wk, np.float32).T),
        "wvT": _bf16(np.asarray(Wv, np.float32).T),
        "wmT": _bf16(np.asarray(Wm, np.float32).T),
        "bq": np.ascontiguousarray(np.asarray(bq, np.float32).reshape(NDT, P).T),
        "bk": np.ascontiguousarray(np.asarray(bk, np.float32).reshape(NDT, P).T),
        "bm": np.ascontiguousarray(np.asarray(bm, np.float32).reshape(NDT, P).T),
        "bv": np.asarray(bv, np.float32).reshape(1, D),
    }
    eye = np.eye(GN, dtype=np.float32)
    in_maps = []
    for b in range(B):
        mb = np.where(mask[b, 0, 0], np.float32(-1e9), np.float32(0.0)).astype(np.float32)
        in_maps.append(
            dict(
                shared,
                qinT=_bf16(q[b].T),
                kinT=_bf16(k[b].T),
                vinT=_bf16(v[b].T),
                maskb=np.ascontiguousarray(mb.reshape(NKT, P).T),
                gT=np.ascontiguousarray((graph[b] + eye).T),
            )
        )

    res = bass_utils.run_bass_kernel_spmd(
        nc, in_maps, core_ids=list(range(B)), trace=_trace
    )
    out = np.stack([r["outT"].T for r in res.results]).astype(np.float32)
    if _trace:
        kernel._last_results = res
    return out



# revision 9
# speedup vs baseline: 1.1645x; 1.0040x over previous
"""Sparse-attention (graph-modulated MHA) Bass kernel for Trainium2.

Strategy: data-parallel over batch (8 batches -> 8 NeuronCores). Per core:
  - bf16 matmuls (fp32 psum); V projection first, then Q/K projections
    interleaved per head-pair with the score matmuls so the ACT-engine exp
    work overlaps projection matmuls on the PE
  - scores computed transposed sT[k_pos, q]; the two heads of a pair share
    one [128, 1024] psum tile so one exp covers both; graph block multiplied
    on raw fp32 psum scores; key mask folded into the exp bias
  - softmax without max-subtraction; denominator L from an extra ones-column
    in the attention*V matmul; reciprocals batched (pairs 0-5 mid-loop, rest
    at the end); 1/L broadcast across partitions via DRAM round-trip DMA
  - merge projection emitted transposed (fp32 out); host transposes back
  - bulk loads on the sync DMA queue; small/late DMAs on the gpsimd queue
"""
import sys

sys.path.insert(0, "/opt/trn_rl_repo")

import ml_dtypes
import numpy as np

import concourse.bass as bass
import concourse.mybir as mybir
import concourse.tile as tile
from concourse import bacc, bass_utils
from concourse.bass import ds, ts

B, S, D, H, DK = 8, 512, 1024, 16, 64
GN = 100
P = 128
NKT = S // P      # 4 key-position chunks of 128
NDT = D // P      # 8 hidden chunks of 128
NPAIR = H // 2    # 8 head pairs (2 heads share a 128-partition tile)
EH = DK + 1       # head slot width in vha (64 v-cols + 1 ones col)
NB1 = 6           # pairs normalized in the first (mid-loop) batch
F32 = mybir.dt.float32
BF16 = mybir.dt.bfloat16
FT = mybir.ActivationFunctionType
ALU = mybir.AluOpType

_CACHE: dict = {}


def _build_module():
    nc = bacc.Bacc("TRN2", target_bir_lowering=False, debug=False)
    dram = {}
    for nm in ("qinT", "kinT", "vinT"):
        dram[nm] = nc.dram_tensor(nm, [D, S], BF16, kind="ExternalInput").ap()
    for nm in ("wqT", "wkT", "wvT", "wmT"):
        dram[nm] = nc.dram_tensor(nm, [D, D], BF16, kind="ExternalInput").ap()
    for nm in ("bq", "bk", "bm"):
        dram[nm] = nc.dram_tensor(nm, [P, NDT], F32, kind="ExternalInput").ap()
    dram["bv"] = nc.dram_tensor("bv", [1, D], F32, kind="ExternalInput").ap()
    dram["maskb"] = nc.dram_tensor("maskb", [P, NKT], F32, kind="ExternalInput").ap()
    dram["gT"] = nc.dram_tensor("gT", [GN, GN], F32, kind="ExternalInput").ap()
    outT = nc.dram_tensor("outT", [D, S], BF16, kind="ExternalOutput").ap()

    with tile.TileContext(nc) as tc:
        with (
            tc.tile_pool(name="wpool", bufs=24) as wpool,
            tc.tile_pool(name="xpool", bufs=24) as xpool,
            tc.tile_pool(name="qkpool", bufs=16) as qkpool,
            tc.tile_pool(name="vpool", bufs=4) as vpool,
            tc.tile_pool(name="ptpool", bufs=14) as ptpool,
            tc.tile_pool(name="opool", bufs=8) as opool,
            tc.tile_pool(name="outpool", bufs=3) as outpool,
            tc.tile_pool(name="cpool", bufs=1) as cpool,
            tc.tile_pool(name="rlpool", bufs=2) as rlpool,
            tc.tile_pool(name="rlbpool", bufs=4) as rlbpool,
            tc.tile_pool(name="drampool", bufs=2, space="DRAM") as drampool,
            tc.tile_pool(name="ppsum", bufs=2, space="PSUM") as ppsum,
            tc.tile_pool(name="spsum", bufs=2, space="PSUM") as spsum,
            tc.tile_pool(name="apsum", bufs=2, space="PSUM") as apsum,
        ):
            def load_chunks(name, width, eng, eng2=None):
                tiles = []
                src = dram[name].rearrange("(t p) f -> t p f", p=P)
                pool = wpool if width == D else xpool
                for k_i in range(NDT):
                    t_ = pool.tile([P, width], BF16, tag="w" if width == D else "x")
                    e = eng if (eng2 is None or k_i % 2 == 0) else eng2
                    e.dma_start(t_[:], src[k_i])
                    tiles.append(t_)
                return tiles

            # PE warmup: ~4us of full-duty N=512 matmuls on memset tiles while
            # the first DMAs land, so the HAM un-throttles before real matmuls
            warm_w = cpool.tile([P, DK], BF16, tag="warmw")
            nc.vector.memset(warm_w[:], 0.0)
            warm_x = cpool.tile([P, S], BF16, tag="warmx")
            nc.vector.memset(warm_x[:], 0.0)
            wps = apsum.tile([EH, S], F32, tag="ap", name="warmps")
            for _ in range(10):
                nc.tensor.matmul(wps[0:DK, :], warm_w[:], warm_x[:], start=True, stop=True)
            for _ in range(28):
                nc.tensor.matmul(wps[0:DK, 0:256], warm_w[:], warm_x[:, 0:256], start=True, stop=True)

            # V inputs stream first (V projection runs first);
            # weights on the sync queue, inputs on the scalar queue
            wvt = load_chunks("wvT", D, nc.sync, nc.gpsimd)
            vt = load_chunks("vinT", S, nc.scalar, nc.sync)

            # ---- constants (gpsimd DMA queue; small) ----
            bqt = cpool.tile([P, NDT], F32, tag="bqt")
            nc.gpsimd.dma_start(bqt[:], dram["bq"])
            bkt = cpool.tile([P, NDT], F32, tag="bkt")
            nc.gpsimd.dma_start(bkt[:], dram["bk"])
            bmt = cpool.tile([P, NDT], F32, tag="bmt")
            nc.gpsimd.dma_start(bmt[:], dram["bm"])
            maskb = cpool.tile([P, NKT], F32, tag="maskb")
            nc.gpsimd.dma_start(maskb[:], dram["maskb"])
            gt = cpool.tile([P, GN], F32, tag="gt")
            nc.gpsimd.dma_start(gt[0:GN, :], dram["gT"])
            bvb = cpool.tile([P, D], F32, tag="bvb")
            nc.gpsimd.dma_start(bvb[:], dram["bv"].to_broadcast((P, D)))
            ones64 = cpool.tile([1, DK], mybir.dt.float32r, tag="ones64")
            nc.vector.memset(ones64[:].bitcast(F32), 1.0)

            # Q/K inputs stream behind V
            wqt = load_chunks("wqT", D, nc.gpsimd)
            qt = load_chunks("qinT", S, nc.scalar)
            wkt = load_chunks("wkT", D, nc.sync)
            ktc = load_chunks("kinT", S, nc.scalar)

            # ---- V projection (natural layout, packed into vha with ones col) ----
            vha = [vpool.tile([P, H * EH], BF16, tag="vha", name=f"vha{i}") for i in range(NKT)]
            for st in range(NKT):
                v3 = vha[st].rearrange("p (h e) -> p h e", e=EH)
                for half in range(2):
                    ps = ppsum.tile([P, S], F32, tag="pp")
                    for k_i in range(NDT):
                        nc.tensor.matmul(
                            ps[:], vt[k_i][:, ts(st, P)], wvt[k_i][:, ts(half, 512)],
                            start=(k_i == 0), stop=(k_i == NDT - 1),
                        )
                    dst3 = v3[:, half * 8 : half * 8 + 8, 0:DK]
                    src3 = ps[:].rearrange("p (h d) -> p h d", d=DK)
                    bv3 = bvb[:, ts(half, 512)].rearrange("p (h d) -> p h d", d=DK)
                    nc.vector.tensor_tensor(dst3, src3, bv3, ALU.add)
                nc.vector.memset(v3[:, :, DK : DK + 1], 1.0)

            # merge weights stream during the attention phase
            wmt = load_chunks("wmT", D, nc.sync)

            # ---- attention state ----
            oT = [opool.tile([P, S], BF16, tag="o", name=f"oT{i}") for i in range(NPAIR)]
            qT, kT = [None] * NDT, [None] * NDT

            def emit_proj(wt, xt, btile, dst, m):
                ps = ppsum.tile([P, S], F32, tag="pp")
                for k_i in range(NDT):
                    nc.tensor.matmul(
                        ps[:], wt[k_i][:, ts(m, P)], xt[k_i][:],
                        start=(k_i == 0), stop=(k_i == NDT - 1),
                    )
                t_ = qkpool.tile([P, S], BF16, tag="qk")
                nc.scalar.activation(
                    t_[:], ps[:], FT.Identity, bias=btile[:, m : m + 1]
                )
                dst[m] = t_

            def emit_scores(t):
                """Both heads of pair t share one [128, 2*S] psum tile per k-chunk."""
                tiles = [None] * NKT
                for kc in range(NKT):
                    sps = spsum.tile([P, 2 * S], F32, tag="sp")
                    for x in range(2):
                        nc.tensor.matmul(
                            sps[:, ts(x, S)],
                            kT[t][x * DK : (x + 1) * DK, ts(kc, P)],
                            qT[t][x * DK : (x + 1) * DK, :],
                            start=True, stop=True,
                        )
                        if kc == 0:
                            nc.vector.tensor_tensor(
                                sps[0:GN, x * S : x * S + GN],
                                sps[0:GN, x * S : x * S + GN],
                                gt[0:GN, :], ALU.mult,
                            )
                    pt = ptpool.tile([P, 2 * S], BF16, tag="pt")
                    nc.scalar.activation(
                        pt[:], sps[:], FT.Exp,
                        bias=maskb[:, kc : kc + 1], scale=0.125,
                    )
                    tiles[kc] = pt
                return tiles

            def emit_av(t, ptiles):
                lrec = rlpool.tile([1, 2 * S], F32, tag="lrec")
                for x in range(2):
                    h = 2 * t + x
                    ops = apsum.tile([EH, S], F32, tag="ap")
                    for kc in range(NKT):
                        nc.tensor.matmul(
                            ops[:], vha[kc][:, ds(h * EH, EH)],
                            ptiles[kc][:, ts(x, S)],
                            start=(kc == 0), stop=(kc == NKT - 1),
                        )
                    lsb_ = rlpool.tile([1, S], F32, tag="lsb")
                    nc.scalar.copy(lsb_[:], ops[DK : DK + 1, :])
                    nc.vector.reciprocal_approx_fast(
                        lrec[0:1, ts(x, S)], lsb_[0:1, :]
                    )
                    nc.vector.tensor_copy(
                        oT[t][x * DK : (x + 1) * DK, :], ops[0:DK, :]
                    )
                rlr = rlbpool.tile([1, 2 * S], mybir.dt.float32r, tag="rlr")
                nc.vector.tensor_copy(rlr[:], lrec[:])
                lba = apsum.tile([EH, S], F32, tag="ap", name=f"lba{t}")
                nc.tensor.matmul(
                    lba[0:DK, :], ones64[:], rlr[0:1, 0:S], start=True, stop=True
                )
                lbb = apsum.tile([EH, S], F32, tag="ap", name=f"lbb{t}")
                nc.tensor.matmul(
                    lbb[0:DK, :], ones64[:], rlr[0:1, ts(1, S)], start=True, stop=True
                )
                oa = oT[t][0:DK, :]
                nc.vector.tensor_tensor(oa, oa, lba[0:DK, :], ALU.mult)
                ob = oT[t][DK:P, :]
                nc.vector.tensor_tensor(ob, ob, lbb[0:DK, :], ALU.mult)

            # ---- merge helpers: kd 0..5 accumulate early, kd 6..7 close late ----
            out_view = outT.rearrange("(t p) f -> t p f", p=P)
            mps = {}

            def merge_start(m):
                if m % 2 == 0:
                    ps = ppsum.tile([P, S], F32, tag="pp", name=f"mps{m}")
                else:
                    ps = spsum.tile([P, 2 * S], F32, tag="sp", name=f"mps{m}")[:, 0:S]
                for k_i in range(NDT - 2):
                    nc.tensor.matmul(
                        ps[:], wmt[k_i][:, ts(m, P)], oT[k_i][:],
                        start=(k_i == 0), stop=False,
                    )
                mps[m] = ps

            def merge_fin(m):
                ps = mps.pop(m)
                for k_i in (NDT - 2, NDT - 1):
                    nc.tensor.matmul(
                        ps[:], wmt[k_i][:, ts(m, P)], oT[k_i][:],
                        start=False, stop=(k_i == NDT - 1),
                    )
                ot = outpool.tile([P, S], BF16, tag="out")
                nc.scalar.activation(
                    ot[:], ps[:], FT.Identity, bias=bmt[:, m : m + 1]
                )
                out_eng = (nc.gpsimd, nc.sync, nc.scalar)[m % 3]
                out_eng.dma_start(out_view[m], ot[:])

            # ---- main interleaved loop ----
            prev = None
            for t in range(NPAIR):
                emit_proj(wqt, qt, bqt, qT, t)
                emit_proj(wkt, ktc, bkt, kT, t)
                cur = emit_scores(t)
                if prev is not None:
                    emit_av(t - 1, prev)
                prev = cur
            merge_start(0)
            emit_av(NPAIR - 1, prev)
            merge_start(1)
            merge_start(2)
            merge_start(3)
            for m in range(NDT):
                merge_fin(m)
                if m + 4 < NDT:
                    merge_start(m + 4)

    nc.compile()
    return nc


def _get_module():
    if "nc" not in _CACHE:
        _CACHE["nc"] = _build_module()
    return _CACHE["nc"]


def _bf16(x: np.ndarray) -> np.ndarray:
    return np.ascontiguousarray(x, dtype=np.float32).astype(ml_dtypes.bfloat16)


def kernel(q, k, v, mask, graph, Wv, bv, Wk, bk, Wq, bq, Wm, bm, _trace=False):
    nc = _get_module()
    q = np.asarray(q, np.float32)
    k = np.asarray(k, np.float32)
    v = np.asarray(v, np.float32)
    mask = np.asarray(mask)
    graph = np.asarray(graph, np.float32)

    shared = {
        "wqT": _bf16(np.asarray(Wq, np.float32).T),
        "wkT": _bf16(np.asarray(Wk, np.float32).T),
        "wvT": _bf16(np.asarray(Wv, np.float32).T),
        "wmT": _bf16(np.asarray(Wm, np.float32).T),
        "bq": np.ascontiguousarray(np.asarray(bq, np.float32).reshape(NDT, P).T),
        "bk": np.ascontiguousarray(np.asarray(bk, np.float32).reshape(NDT, P).T),
        "bm": np.ascontiguousarray(np.asarray(bm, np.float32).reshape(NDT, P).T),
        "bv": np.asarray(bv, np.float32).reshape(1, D),
    }
    eye = np.eye(GN, dtype=np.float32)
    in_maps = []
    for b in range(B):
        mb = np.where(mask[b, 0, 0], np.float32(-1e9), np.float32(0.0)).astype(np.float32)
        in_maps.append(
            dict(
                shared,
                qinT=_bf16(q[b].T),
                kinT=_bf16(k[b].T),
                vinT=_bf16(v[b].T),
                maskb=np.ascontiguousarray(mb.reshape(NKT, P).T),
                gT=np.ascontiguousarray((graph[b] + eye).T),
            )
        )

    res = bass_utils.run_bass_kernel_spmd(
        nc, in_maps, core_ids=list(range(B)), trace=_trace
    )
    out = np.stack([r["outT"].T for r in res.results]).astype(np.float32)
    if _trace:
        kernel._last_results = res
    return out

